# revision 48
# baseline (speedup 1.0000x reference)
"""Trainium2 Bass kernel for causal self-attention + out-proj + residual + LayerNorm.

Sharding: heads (tensor-parallel) across 8 cores for QKV+attention (kernel A),
then sequence-parallel across 8 cores for out-proj + residual + LN (kernel B).

Kernel A (per core, 2 heads), fused per 512-token q-chunk:
 - x.T pre-transposed and fp8-cast on the host; QKV projection runs as fp8
   DoubleRow matmuls over kd-pairs (weights x64 host-scaled against fp8
   subnormals, un-done by the ACT 1/64 scale). V is PE-transposed to
   token-major fp8 with a fused ones column per head.
 - Scores: per-head bf16 matmuls auto-row-tiled to the two array halves
   (concurrent). Joint [128,1024] PSUM score tiles, 3-deep.
 - exp alternates whole [128,1024] steps between ACT (true exp -> fp8) and
   DVE (Schraudolph fp8 bit trick: bitcast(int8(s*8*log2e + 55.4))).
 - Causal masks on GpSimd (fp8 trimask multiply); PV runs as fp8 DoubleRow
   matmuls over adjacent k-tile pairs with the ones column emitting the
   softmax denominator (M=65). Diagonal (masked) pairs are interleaved early
   in each chunk so their mask latency hides behind unmasked PV work.
 - Chunk-end normalize: pv evacuated to SBUF immediately (single-buffered pv
   banks), denominator broadcast via PE ones-matmul (gpsimd
   partition_broadcast stalls 6-8us), reciprocal+scale on DVE, at exported
   fp8.
Kernel B: out-proj as fp8 DoubleRow (at/wout fp8, x64 scaling absorbed by LN
via eps*64^2), residual+LN with bf16 z, fp32 stats, (z-mu)*rstd on ACT
(scale/bias APs), gamma/beta loaded [1,D] and PE-broadcast. Whole-tensor
DMAs split across the sync/ACT queues.
"""

import math
from contextlib import ExitStack

import numpy as np
import ml_dtypes

import concourse.bass as bass
import concourse.tile as tile
from concourse import bacc, mybir
from concourse.bass_utils import run_bass_kernel_spmd

# NTFF-trace shim: make run_bass_kernel_spmd(trace=True) usable in containers
# whose antenv lacks axon_hooks (harmless when tracing is off).
def _install_trace_shim():
    import sys, types
    try:
        import antenv.axon_hooks  # noqa: F401
        return
    except ImportError:
        pass
    try:
        import antenv
        from trn_agent_boot.trn_boot import _ntff_profile_via_ctypes
        hook = _ntff_profile_via_ctypes("/opt/axon/libaxon_pjrt.so")
        mod = types.ModuleType("antenv.axon_hooks")
        mod.get_axon_ntff_profile_hook = lambda: hook
        mod.set_axon_ntff_profile_hook = lambda h: None
        sys.modules["antenv.axon_hooks"] = mod
        antenv.axon_hooks = mod
        import concourse.bass_utils as _bu
        _bu.upload_artifacts = lambda tmpdir: "local://skipped"
    except Exception:
        pass


_install_trace_shim()

F32 = mybir.dt.float32
BF16 = mybir.dt.bfloat16
I16 = mybir.dt.int16
I8 = mybir.dt.int8
FP8 = mybir.dt.float8e4
DR = mybir.MatmulPerfMode.DoubleRow
EXP = mybir.ActivationFunctionType.Exp
IDENT_FN = mybir.ActivationFunctionType.Identity
SQRT = mybir.ActivationFunctionType.Sqrt
BF = ml_dtypes.bfloat16

T_FULL = 4096
D = 1024
HEADS = 16
NCORES = 8
LN_EPS = 1e-5

# Schraudolph bf16 exp constants (validated on HW: DVE rounds to nearest)
LOG2E = 1.4426950408889634
SCH_A = 128.0 * LOG2E
SCH_B = 127.0 * 128.0 - 6.0
# Schraudolph fp8e4m3 variant: bits = s*8*log2e + 7*8 - c. Valid (no sign
# wrap / no inf) for scores s in (-4.8, +6.1); actual |s| <= ~2.5 here.
SCH_A8 = 8.0 * LOG2E
SCH_B8 = 7.0 * 8.0 - 0.6

_CACHE = {}
LAST_RESULTS = {}


def build_kernel_a(T=T_FULL):
    """Per core: 2 heads. Computes A.T = softmax(QK^T/sqrt(d)) @ V, transposed
    ([128 = 2*64 head dims, T] bf16) and normalized."""
    nc = bacc.Bacc("TRN2", target_bir_lowering=False, debug=False)
    KD = D // 128          # 8 contraction tiles over D
    NT = T // 128          # token tiles of 128
    NQ = T // 512          # query chunks of 512

    xt_d = nc.dram_tensor("xt", [128, KD, T], FP8, kind="ExternalInput")
    id_d = nc.dram_tensor("ident", [128, 128], BF16, kind="ExternalInput")
    tm_d = nc.dram_tensor("trimask", [128, 128], FP8, kind="ExternalInput")
    wq_d = nc.dram_tensor("wq_t", [128, KD, 128], FP8, kind="ExternalInput")
    wk_d = nc.dram_tensor("wk_t", [128, KD, 128], FP8, kind="ExternalInput")
    wv_d = nc.dram_tensor("wv_t", [128, KD, 128], FP8, kind="ExternalInput")
    bq_d = nc.dram_tensor("bq", [128, 1], F32, kind="ExternalInput")
    bk_d = nc.dram_tensor("bk", [128, 1], F32, kind="ExternalInput")
    at_d = nc.dram_tensor("at_out", [128, T], FP8, kind="ExternalOutput")

    with tile.TileContext(nc) as tc, ExitStack() as ctx:
        const = ctx.enter_context(tc.tile_pool(name="const", bufs=1))
        persist = ctx.enter_context(tc.tile_pool(name="persist", bufs=1))
        vtp = ctx.enter_context(tc.tile_pool(name="vtp", bufs=2))
        e_pool = ctx.enter_context(tc.tile_pool(name="e_pool", bufs=8))
        rb_pool = ctx.enter_context(tc.tile_pool(name="rb_pool", bufs=2))
        # PSUM budget (8 banks of 2KB): joint score tiles [128,1024] f32
        # (2 banks) x 3 bufs = 6 banks, pv0/pv1 x 1 buf = 2 banks (the
        # chunk-end normalize evacuates pv to SBUF immediately so the next
        # chunk's PV accumulation is not blocked). The projection matmuls,
        # V transposes and denominator broadcasts borrow the score slots.
        s_ps = ctx.enter_context(tc.tile_pool(name="s_ps", bufs=3, space="PSUM"))
        pv_ps = ctx.enter_context(tc.tile_pool(name="pv_ps", bufs=1, space="PSUM"))

        # DMA order matters for the cold start: ident (warm-up exp input),
        # q/k weights + chunk-0 xt (first proj), then the rest.
        ident = const.tile([128, 128], BF16)
        nc.sync.dma_start(ident[:], id_d.ap())
        wq_sb = const.tile([128, KD, 128], FP8, tag="wq")
        wk_sb = const.tile([128, KD, 128], FP8, tag="wk")
        wv_sb = const.tile([128, KD, 128], FP8, tag="wv")
        nc.sync.dma_start(wq_sb[:], wq_d.ap())
        nc.sync.dma_start(wk_sb[:], wk_d.ap())
        bq_sb = const.tile([128, 1], F32, tag="bq")
        bk_sb = const.tile([128, 1], F32, tag="bk")
        xt_sb = persist.tile([128, KD, T], FP8, tag="xt")
        nc.sync.dma_start(xt_sb[:, :, 0:512], xt_d.ap()[:, :, 0:512])
        nc.sync.dma_start(wv_sb[:], wv_d.ap())
        nc.sync.dma_start(bq_sb[:], bq_d.ap())
        nc.sync.dma_start(bk_sb[:], bk_d.ap())
        trimask = const.tile([128, 128], FP8)
        nc.sync.dma_start(trimask[:], tm_d.ap())
        for vc in range(1, NQ):
            v_sl = slice(vc * 512, (vc + 1) * 512)
            nc.sync.dma_start(xt_sb[:, :, v_sl], xt_d.ap()[:, :, v_sl])
        # Warm the ACT exp table during the initial DMAs (first real EXP
        # otherwise pays a ~1.3us ACT_TABLE_LOAD on the critical path).
        # Reads ident (small, first DMA in queue) so it runs immediately.
        warm = const.tile([128, 1], F32, tag="warm")
        nc.scalar.activation(out=warm[:], in_=ident[:, 0:1], func=EXP)
        # Ones row for the PE-based denominator broadcast (gpsimd
        # partition_broadcast stalls ~6-8us on library reload / sem polling).
        ones_r = const.tile([1, 128], BF16, tag="ones_r")
        nc.vector.memset(ones_r[:], 1.0)
        # Warm-up matmuls on ident while the xt/weight DMAs stream: the PE
        # is idle here anyway, and ramping HAM early makes the first proj
        # matmuls run at full clock instead of the cold ~0.65GHz.
        for w in range(24):
            wps = s_ps.tile([128, 128], F32, tag="s", name=f"warm_mm_{w}")
            nc.tensor.matmul(wps[:], ident[:], ident[:], start=True, stop=True)

        # V in fp8 [k-token part, kt, 144(pad for DoubleRow step%16)]: per
        # head 64 V cols + a ones col (65th) that makes the PV matmul emit
        # the softmax denominator. (V bias is folded into kernel B's xb on
        # the host: softmax weights sum to one, so a constant v-offset
        # shifts the output by a constant.)
        v_sb = persist.tile([128, NT, 144], FP8, tag="v")
        nc.gpsimd.memset(v_sb[:, :, 64:65], 1.0)
        nc.gpsimd.memset(v_sb[:, :, 129:130], 1.0)
        qt_sb = persist.tile([128, T], BF16, tag="qt")
        kt_sb = persist.tile([128, T], BF16, tag="kt")
        at_sb = persist.tile([128, T], FP8, tag="at")

        def proj_chains(vc):
            """QKV projection (fp8 DoubleRow over kd-pairs; weights are x64
            host-scaled against fp8 subnormals, un-done by the ACT scale) +
            V-transpose for chunk vc, as 4 separately-emittable chains that
            get spread through the previous chunk's attention steps: the PE
            queue is in-order, so interleaved proj work fills exp-wait slots
            and keeps the HAM p-state high. PSUM tiles borrow score slots."""
            c_sl = slice(vc * 512, (vc + 1) * 512)
            vt_cell = []

            def qk_chain(nm, w_sb, b_sb, o_sb):
                def f():
                    pps = s_ps.tile([128, 512], F32, tag="s",
                                    name=f"pps_{nm}_{vc}")
                    for j in range(KD // 2):
                        nc.tensor.matmul(pps[:], w_sb[:, 2 * j:2 * j + 2, :],
                                         xt_sb[:, 2 * j:2 * j + 2, c_sl],
                                         start=(j == 0),
                                         stop=(j == KD // 2 - 1),
                                         perf_mode=DR)
                    nc.scalar.activation(out=o_sb[:, c_sl], in_=pps[:],
                                         func=IDENT_FN, bias=b_sb[:],
                                         scale=1.0 / 64.0)
                return f

            def v_chain():
                vps = s_ps.tile([128, 512], F32, tag="s", name=f"vps_{vc}")
                for j in range(KD // 2):
                    nc.tensor.matmul(vps[:], wv_sb[:, 2 * j:2 * j + 2, :],
                                     xt_sb[:, 2 * j:2 * j + 2, c_sl],
                                     start=(j == 0), stop=(j == KD // 2 - 1),
                                     perf_mode=DR)
                vt_c = vtp.tile([128, 512], BF16, tag="vt", name=f"vt_{vc}")
                nc.scalar.activation(out=vt_c[:], in_=vps[:], func=IDENT_FN,
                                     scale=1.0 / 64.0)
                vt_cell.append(vt_c)

            def tr_chain():
                vt_c = vt_cell[0]
                tpv = s_ps.tile([128, 4, 128], BF16, tag="s", name=f"tpv_{vc}")
                for q in range(4):
                    nc.tensor.transpose(tpv[:, q, :],
                                        vt_c[:, q * 128:(q + 1) * 128],
                                        ident[:])
                nc.vector.tensor_copy(v_sb[:, vc * 4:(vc + 1) * 4, 0:64],
                                      tpv[:, :, 0:64])
                nc.vector.tensor_copy(v_sb[:, vc * 4:(vc + 1) * 4, 65:129],
                                      tpv[:, :, 64:128])

            return [qk_chain("q", wq_sb, bq_sb, qt_sb),
                    qk_chain("k", wk_sb, bk_sb, kt_sb),
                    v_chain, tr_chain]

        def emit_proj(vc):
            for f in proj_chains(vc):
                f()

        def mask_cols(esb, kt, qc):
            """Causal mask for diagonal k-tile kt on the [128, 1024] fp8 exp
            tile (cols 512h..: head h): zero fully-masked leading columns,
            multiply the 128-wide diagonal block by the triangular mask."""
            o = kt * 128 - qc * 512
            for h in (0, 1):
                if o > 0:
                    nc.gpsimd.memset(esb[:, 512 * h:512 * h + o], 0.0)
                nc.gpsimd.tensor_mul(esb[:, 512 * h + o:512 * h + o + 128],
                                     esb[:, 512 * h + o:512 * h + o + 128],
                                     trimask[:])

        emit_proj(0)
        for qc in range(NQ):
            c_sl = slice(qc * 512, (qc + 1) * 512)
            # ---- attention for chunk qc ----
            nkt = 4 * (qc + 1)
            npair = nkt // 2
            q_sl = c_sl
            pv = [pv_ps.tile([65, 512], F32, tag=f"pv{h}", name=f"pv{h}_{qc}")
                  for h in (0, 1)]

            # Pair-preserving interleave: PV runs as fp8 DoubleRow matmuls
            # over adjacent k-tile pairs (2i, 2i+1), so pairs stay intact.
            # The 2 masked diagonal pairs are spread early among the unmasked
            # ones (u0 d0 u1 d1 u2 u3 ...): their long exp->gpsimd-mask
            # latency hides behind unmasked PV work instead of draining the
            # pipeline at the chunk tail.
            dpairs = [npair - 2, npair - 1]
            upairs = list(range(npair - 2))
            pseq = []
            for i in range(2):
                if i < len(upairs):
                    pseq.append(upairs[i])
                pseq.append(dpairs[i])
            pseq += upairs[2:]
            seq = [2 * p + s for p in pseq for s in (0, 1)]

            n_emitted = 0

            def emit_pv(pair, esb):
                """One DoubleRow matmul per head covers k-tiles 2p and 2p+1:
                lhsT [128, 2, 65] fp8 (two V tiles), rhs [128, 2, 512] fp8
                (two exp tiles interleaved in the free dim)."""
                nonlocal n_emitted
                for h in (0, 1):
                    nc.tensor.matmul(pv[h][:, :],
                                     v_sb[:, 2 * pair:2 * pair + 2,
                                          65 * h:65 * h + 65],
                                     esb[:, :, 512 * h:512 * h + 512],
                                     start=(n_emitted == 0),
                                     stop=(n_emitted == npair - 1),
                                     perf_mode=DR,
                                     skip_group_check=True)
                n_emitted += 1

            esbs = {}
            done_pairs = []
            n_exp = 0
            for j, kt in enumerate(seq):
                if j == 2 and qc + 1 < NQ:
                    emit_proj(qc + 1)
                pair, slot = kt // 2, kt % 2
                is_diag = kt >= nkt - 4
                sp = s_ps.tile([128, 1024], F32, tag="s", name=f"s_{qc}_{kt}")
                for h in (0, 1):
                    h_sl = slice(64 * h, 64 * h + 64)
                    nc.tensor.matmul(sp[:, 512 * h:512 * h + 512],
                                     kt_sb[h_sl, kt * 128:(kt + 1) * 128],
                                     qt_sb[h_sl, q_sl],
                                     start=True, stop=True)
                if slot == 0:
                    esbs[pair] = e_pool.tile([128, 2, 1024], FP8, tag="e",
                                             name=f"e_{qc}_{pair}")
                esb = esbs[pair]
                # Whole-step exp alternating between ACT (exact exp, ~1.03us
                # for 1024 cols) and DVE (Schraudolph fp8 bit-trick, ~1.2us):
                # one instruction per step amortizes the fixed access
                # overhead, ~25% more exp throughput than per-half splitting.
                # (GpSimd cannot read PSUM, so it can't take exp steps.)
                n_exp += 1
                if n_exp % 2 == 1:
                    nc.scalar.activation(out=esb[:, slot, :], in_=sp[:],
                                         func=EXP)
                else:
                    nc.vector.tensor_scalar(out=esb[:, slot, :].bitcast(I8),
                                            in0=sp[:],
                                            scalar1=SCH_A8, scalar2=SCH_B8,
                                            op0=mybir.AluOpType.mult,
                                            op1=mybir.AluOpType.add)
                if is_diag:
                    mask_cols(esb[:, slot, :], kt, qc)
                # after finishing pair m (odd positions), emit PV for the
                # previous pair (one-pair lag to keep the pipeline deep)
                if slot == 1:
                    done_pairs.append(pair)
                    if len(done_pairs) > 2:
                        p = done_pairs.pop(0)
                        emit_pv(p, esbs.pop(p))
            for p in done_pairs:
                emit_pv(p, esbs.pop(p))

            # Evacuate pv immediately (r1 on ACT, dims on DVE) so the
            # single-buffered pv banks free up for the next chunk's PVs.
            for h in (0, 1):
                r1 = rb_pool.tile([1, 512], BF16, tag="r1", name=f"r1{h}_{qc}")
                nc.scalar.copy(r1[:], pv[h][64:65, :])
                pvc = rb_pool.tile([64, 512], F32, tag="pvc", name=f"pvc{h}_{qc}")
                nc.vector.tensor_copy(pvc[:], pv[h][0:64, :])
                rb = s_ps.tile([128, 512], F32, tag="s", name=f"rb{h}_{qc}")
                nc.tensor.matmul(rb[:], ones_r[:], r1[:], start=True, stop=True)
                rbs = rb_pool.tile([128, 512], F32, tag="rbs", name=f"rbs{h}_{qc}")
                nc.vector.reciprocal_approx_fast(out=rbs[:], in_=rb[:])
                nc.vector.tensor_mul(at_sb[64 * h:64 * h + 64, q_sl],
                                     pvc[:], rbs[0:64, :])
            nc.sync.dma_start(at_d.ap()[:, q_sl], at_sb[:, q_sl])

    nc.compile()
    return nc


def build_kernel_b(T=T_FULL):
    """Per core: rows slice of T/8 tokens: out-proj (bf16) + residual (+bout
    folded on host into xb, sent bf16) + LayerNorm*gamma+beta. DMAs are split
    per kd-tile and interleaved so the first matmul starts ~2us in; z is kept
    bf16 (2x DVE rate), LN stats fp32, the (z-mu)*rstd normalize runs on the
    ACT engine (scale/bias APs), gamma/beta loaded [1,D] and broadcast via a
    PE ones-matmul. gpsimd untouched (its teardown drain costs ~50us)."""
    nc = bacc.Bacc("TRN2", target_bir_lowering=False, debug=False)
    Tc = T // NCORES
    KD = D // 128

    at_d = nc.dram_tensor("at", [128, KD, Tc], FP8, kind="ExternalInput")
    wo_d = nc.dram_tensor("wout_t", [128, KD, D], FP8, kind="ExternalInput")
    xb_d = nc.dram_tensor("xb", [Tc, D], BF16, kind="ExternalInput")
    g_d = nc.dram_tensor("gamma", [1, D], BF16, kind="ExternalInput")
    be_d = nc.dram_tensor("beta", [1, D], BF16, kind="ExternalInput")
    y_d = nc.dram_tensor("y", [Tc, D], F32, kind="ExternalOutput")

    with tile.TileContext(nc) as tc, ExitStack() as ctx:
        const = ctx.enter_context(tc.tile_pool(name="const", bufs=1))
        work = ctx.enter_context(tc.tile_pool(name="work", bufs=2))
        stats = ctx.enter_context(tc.tile_pool(name="stats", bufs=4))
        ps = ctx.enter_context(tc.tile_pool(name="ps", bufs=4, space="PSUM"))
        gb_ps = ctx.enter_context(tc.tile_pool(name="gb_ps", bufs=2, space="PSUM"))

        # Whole-tensor DMAs on separate engine queues: small kd-split DMAs
        # serialized descriptor generation on the sync sequencer and delayed
        # the first matmul by ~6us.
        at_sb = const.tile([128, KD, Tc], FP8, tag="at")
        wo_sb = const.tile([128, KD, D], FP8, tag="wo")
        gb_sb = const.tile([1, 2, D], BF16, tag="gb")
        nc.sync.dma_start(gb_sb[:, 0, :], g_d.ap())
        nc.sync.dma_start(gb_sb[:, 1, :], be_d.ap())
        nc.sync.dma_start(at_sb[:], at_d.ap())
        nc.scalar.dma_start(wo_sb[:], wo_d.ap())
        eps_sb = const.tile([128, 1], F32, tag="eps")
        # z is x64-scaled (fp8 weights); LN is scale-invariant given eps*64^2
        nc.vector.memset(eps_sb[:], LN_EPS * 4096.0)
        ones_r = const.tile([1, 128], BF16, tag="ones_r")
        nc.vector.memset(ones_r[:], 1.0)
        gam_b = const.tile([128, D], BF16, tag="gam")
        bet_b = const.tile([128, D], BF16, tag="bet")

        for w in range(16):
            wp = gb_ps.tile([128, 512], F32, tag="gp", name=f"warm_b_{w}")
            nc.tensor.matmul(wp[:], ones_r[:], gb_sb[:, 0, 0:512],
                             start=True, stop=True)

        def emit_gb_broadcast():
            # broadcast gamma/beta [1,D] -> [128,D] via PE ones-matmul;
            # emitted after tile 0's matmuls so it doesn't hog the cold PE
            for src, dst in ((0, gam_b), (1, bet_b)):
                for j in (0, 1):
                    gp = gb_ps.tile([128, 512], F32, tag="gp",
                                    name=f"gp_{src}_{j}")
                    nc.tensor.matmul(gp[:], ones_r[:],
                                     gb_sb[:, src, j * 512:(j + 1) * 512],
                                     start=True, stop=True)
                    nc.vector.tensor_copy(dst[:, j * 512:(j + 1) * 512], gp[:])

        for tt in range(Tc // 128):
            t_sl = slice(tt * 128, (tt + 1) * 128)
            xb_t = work.tile([128, D], BF16, tag="xb")
            nc.scalar.dma_start(xb_t[:], xb_d.ap()[t_sl, :])
            z_t = work.tile([128, D], BF16, tag="z")
            st = stats.tile([128, 2, 6], F32, tag="st")
            for j in (0, 1):
                pp = ps.tile([128, 512], F32, tag="pp")
                for kt in range(KD // 2):
                    nc.tensor.matmul(pp[:], at_sb[:, 2 * kt:2 * kt + 2, t_sl],
                                     wo_sb[:, 2 * kt:2 * kt + 2,
                                           j * 512:(j + 1) * 512],
                                     start=(kt == 0), stop=(kt == KD // 2 - 1),
                                     perf_mode=DR)
                nc.vector.tensor_add(z_t[:, j * 512:(j + 1) * 512], pp[:],
                                     xb_t[:, j * 512:(j + 1) * 512])
                nc.vector.bn_stats(st[:, j, :], z_t[:, j * 512:(j + 1) * 512])
            if tt == 0:
                emit_gb_broadcast()
            mv = stats.tile([128, 2], F32, tag="mv")
            nc.vector.bn_aggr(mv[:], st[:])
            sq = stats.tile([128, 1], F32, tag="sq")
            nc.scalar.activation(out=sq[:], in_=mv[:, 1:2], func=SQRT,
                                 bias=eps_sb[:], scale=1.0)
            rstd = stats.tile([128, 1], F32, tag="rstd")
            nc.vector.reciprocal(rstd[:], sq[:])
            nmu = stats.tile([128, 1], F32, tag="nmu")
            nc.vector.tensor_scalar(out=nmu[:], in0=mv[:, 0:1], scalar1=rstd[:],
                                    scalar2=-1.0, op0=mybir.AluOpType.mult,
                                    op1=mybir.AluOpType.mult)
            # (z - mu) * rstd on ACT: func(in*scale + bias)
            zn_t = work.tile([128, D], BF16, tag="zn")
            nc.scalar.activation(out=zn_t[:], in_=z_t[:], func=IDENT_FN,
                                 bias=nmu[:], scale=rstd[:])
            y_t = work.tile([128, D], F32, tag="y")
            nc.vector.tensor_mul(zn_t[:], zn_t[:], gam_b[:])
            nc.vector.tensor_add(y_t[:], zn_t[:], bet_b[:])
            nc.sync.dma_start(y_d.ap()[t_sl, :], y_t[:])

    nc.compile()
    return nc


def _get_kernels(T=T_FULL):
    if T not in _CACHE:
        _CACHE[T] = (build_kernel_a(T), build_kernel_b(T))
    return _CACHE[T]


def _tile_kd(a):
    """[D, M] -> [128, D//128, M] with row = kt*128 + p."""
    Dd, M = a.shape
    return np.ascontiguousarray(a.reshape(Dd // 128, 128, M).transpose(1, 0, 2))


def kernel(x, Wqkv, bqkv, Wout, bout, gamma, beta):
    x = np.asarray(x, dtype=np.float32)
    Wqkv = np.asarray(Wqkv, dtype=np.float32)
    bqkv = np.asarray(bqkv, dtype=np.float32)
    Wout = np.asarray(Wout, dtype=np.float32)
    bout = np.asarray(bout, dtype=np.float32)
    gamma = np.asarray(gamma, dtype=np.float32)
    beta = np.asarray(beta, dtype=np.float32)

    B, T, D_ = x.shape
    assert B == 1 and D_ == D
    d = D // HEADS
    scale = d ** -0.5
    x2d = np.ascontiguousarray(x[0])
    ident = np.eye(128, dtype=np.float32).astype(BF)
    tri = np.triu(np.ones((128, 128), np.float32)).astype(ml_dtypes.float8_e4m3fn)

    nc_a, nc_b = _get_kernels(T)

    F8 = ml_dtypes.float8_e4m3fn

    def q8(a):
        return np.clip(a, -240, 240).astype(F8)

    xt = _tile_kd(q8(x2d.T))                   # [128, 8, T] fp8
    in_maps_a = []
    for c in range(NCORES):
        r = slice(c * 128, (c + 1) * 128)
        wq = Wqkv[0 * D:1 * D][r]
        wk = Wqkv[1 * D:2 * D][r] * scale
        wv = Wqkv[2 * D:3 * D][r]
        in_maps_a.append({
            "xt": xt,
            "ident": ident,
            "trimask": tri,
            # x64 against fp8 subnormals; undone by the ACT 1/64 scale
            "wq_t": _tile_kd(q8(wq.T * 64.0)),
            "wk_t": _tile_kd(q8(wk.T * 64.0)),
            "wv_t": _tile_kd(q8(wv.T * 64.0)),
            "bq": np.ascontiguousarray(bqkv[0 * D:1 * D][r].reshape(128, 1)),
            "bk": np.ascontiguousarray((bqkv[1 * D:2 * D][r] * scale).reshape(128, 1)),
        })
    res_a = run_bass_kernel_spmd(nc_a, in_maps_a, core_ids=list(range(NCORES)))
    LAST_RESULTS["a"] = res_a
    at_full = np.concatenate([np.asarray(res_a.results[c]["at_out"])
                              for c in range(NCORES)], axis=0)  # [D, T] bf16

    Tc = T // NCORES
    wo_tiled = _tile_kd(q8(Wout.T * 64.0))     # [128, 8, D] fp8, x64-scaled
    gam_r = np.ascontiguousarray(gamma.reshape(1, D).astype(BF))
    bet_r = np.ascontiguousarray(beta.reshape(1, D).astype(BF))
    # residual + out-proj bias + folded V bias (constant shift of attention
    # out); x64 to match the fp8 weight scaling (LN un-scales exactly)
    xb_add = bout + Wout @ bqkv[2 * D:3 * D]
    in_maps_b = []
    for c in range(NCORES):
        t_sl = slice(c * Tc, (c + 1) * Tc)
        at_c = at_full[:, t_sl]                # [D, Tc] fp8
        in_maps_b.append({
            "at": _tile_kd(at_c),
            "wout_t": wo_tiled,
            "xb": np.ascontiguousarray(
                ((x2d[t_sl] + xb_add[None, :]) * 64.0).astype(BF)),
            "gamma": gam_r,
            "beta": bet_r,
        })
    res_b = run_bass_kernel_spmd(nc_b, in_maps_b, core_ids=list(range(NCORES)))
    LAST_RESULTS["b"] = res_b
    y = np.concatenate([res_b.results[c]["y"] for c in range(NCORES)], axis=0)
    return y.reshape(1, T, D).astype(np.float32)



# revision 49
# speedup vs baseline: 1.0263x; 1.0263x over previous
"""Trainium2 Bass kernel for causal self-attention + out-proj + residual + LayerNorm.

Sharding: heads (tensor-parallel) across 8 cores for QKV+attention (kernel A),
then sequence-parallel across 8 cores for out-proj + residual + LN (kernel B).

Kernel A (per core, 2 heads), fused per 512-token q-chunk:
 - x.T pre-transposed and fp8-cast on the host; QKV projection runs as fp8
   DoubleRow matmuls over kd-pairs (weights x64 host-scaled against fp8
   subnormals, un-done by the ACT 1/64 scale). V is PE-transposed to
   token-major fp8 with a fused ones column per head.
 - Scores: per-head bf16 matmuls auto-row-tiled to the two array halves
   (concurrent). Joint [128,1024] PSUM score tiles, 3-deep.
 - exp alternates whole [128,1024] steps between ACT (true exp -> fp8) and
   DVE (Schraudolph fp8 bit trick: bitcast(int8(s*8*log2e + 55.4))).
 - Causal masks on GpSimd (fp8 trimask multiply); PV runs as fp8 DoubleRow
   matmuls over adjacent k-tile pairs with the ones column emitting the
   softmax denominator (M=65). Diagonal (masked) pairs are interleaved early
   in each chunk so their mask latency hides behind unmasked PV work.
 - Chunk-end normalize: pv evacuated to SBUF immediately (single-buffered pv
   banks), denominator broadcast via PE ones-matmul (gpsimd
   partition_broadcast stalls 6-8us), reciprocal+scale on DVE, at exported
   fp8.
Kernel B: out-proj as fp8 DoubleRow (at/wout fp8, x64 scaling absorbed by LN
via eps*64^2), residual+LN with bf16 z, fp32 stats, (z-mu)*rstd on ACT
(scale/bias APs), gamma/beta loaded [1,D] and PE-broadcast. Whole-tensor
DMAs split across the sync/ACT queues.
"""

import math
from contextlib import ExitStack

import numpy as np
import ml_dtypes

import concourse.bass as bass
import concourse.tile as tile
from concourse import bacc, mybir
from concourse.bass_utils import run_bass_kernel_spmd

# NTFF-trace shim: make run_bass_kernel_spmd(trace=True) usable in containers
# whose antenv lacks axon_hooks (harmless when tracing is off).
def _install_trace_shim():
    import sys, types
    try:
        import antenv.axon_hooks  # noqa: F401
        return
    except ImportError:
        pass
    try:
        import antenv
        from trn_agent_boot.trn_boot import _ntff_profile_via_ctypes
        hook = _ntff_profile_via_ctypes("/opt/axon/libaxon_pjrt.so")
        mod = types.ModuleType("antenv.axon_hooks")
        mod.get_axon_ntff_profile_hook = lambda: hook
        mod.set_axon_ntff_profile_hook = lambda h: None
        sys.modules["antenv.axon_hooks"] = mod
        antenv.axon_hooks = mod
        import concourse.bass_utils as _bu
        _bu.upload_artifacts = lambda tmpdir: "local://skipped"
    except Exception:
        pass


_install_trace_shim()

F32 = mybir.dt.float32
BF16 = mybir.dt.bfloat16
I16 = mybir.dt.int16
I8 = mybir.dt.int8
FP8 = mybir.dt.float8e4
DR = mybir.MatmulPerfMode.DoubleRow
EXP = mybir.ActivationFunctionType.Exp
IDENT_FN = mybir.ActivationFunctionType.Identity
SQRT = mybir.ActivationFunctionType.Sqrt
BF = ml_dtypes.bfloat16

T_FULL = 4096
D = 1024
HEADS = 16
NCORES = 8
LN_EPS = 1e-5

# Schraudolph bf16 exp constants (validated on HW: DVE rounds to nearest)
LOG2E = 1.4426950408889634
SCH_A = 128.0 * LOG2E
SCH_B = 127.0 * 128.0 - 6.0
# Schraudolph fp8e4m3 variant: bits = s*8*log2e + 7*8 - c. Valid (no sign
# wrap / no inf) for scores s in (-4.8, +6.1); actual |s| <= ~2.5 here.
SCH_A8 = 8.0 * LOG2E
SCH_B8 = 7.0 * 8.0 - 0.6

_CACHE = {}
LAST_RESULTS = {}


def build_kernel_a(T=T_FULL):
    """Per core: 2 heads. Computes A.T = softmax(QK^T/sqrt(d)) @ V, transposed
    ([128 = 2*64 head dims, T] bf16) and normalized."""
    nc = bacc.Bacc("TRN2", target_bir_lowering=False, debug=False)
    KD = D // 128          # 8 contraction tiles over D
    NT = T // 128          # token tiles of 128
    NQ = T // 512          # query chunks of 512

    xt_d = nc.dram_tensor("xt", [128, KD, T], FP8, kind="ExternalInput")
    id_d = nc.dram_tensor("ident", [128, 128], BF16, kind="ExternalInput")
    tm_d = nc.dram_tensor("trimask", [128, 128], FP8, kind="ExternalInput")
    wq_d = nc.dram_tensor("wq_t", [128, KD, 128], FP8, kind="ExternalInput")
    wk_d = nc.dram_tensor("wk_t", [128, KD, 128], FP8, kind="ExternalInput")
    wv_d = nc.dram_tensor("wv_t", [128, KD, 128], FP8, kind="ExternalInput")
    bq_d = nc.dram_tensor("bq", [128, 1], F32, kind="ExternalInput")
    bk_d = nc.dram_tensor("bk", [128, 1], F32, kind="ExternalInput")
    at_d = nc.dram_tensor("at_out", [128, T], FP8, kind="ExternalOutput")

    with tile.TileContext(nc) as tc, ExitStack() as ctx:
        const = ctx.enter_context(tc.tile_pool(name="const", bufs=1))
        persist = ctx.enter_context(tc.tile_pool(name="persist", bufs=1))
        vtp = ctx.enter_context(tc.tile_pool(name="vtp", bufs=2))
        e_pool = ctx.enter_context(tc.tile_pool(name="e_pool", bufs=8))
        rb_pool = ctx.enter_context(tc.tile_pool(name="rb_pool", bufs=2))
        # PSUM budget (8 banks of 2KB): joint score tiles [128,1024] f32
        # (2 banks) x 3 bufs = 6 banks, pv0/pv1 x 1 buf = 2 banks (the
        # chunk-end normalize evacuates pv to SBUF immediately so the next
        # chunk's PV accumulation is not blocked). The projection matmuls,
        # V transposes and denominator broadcasts borrow the score slots.
        s_ps = ctx.enter_context(tc.tile_pool(name="s_ps", bufs=3, space="PSUM"))
        pv_ps = ctx.enter_context(tc.tile_pool(name="pv_ps", bufs=1, space="PSUM"))

        # DMA order matters for the cold start: ident (warm-up exp input),
        # q/k weights + chunk-0 xt (first proj), then the rest.
        ident = const.tile([128, 128], BF16)
        nc.sync.dma_start(ident[:], id_d.ap())
        wq_sb = const.tile([128, KD, 128], FP8, tag="wq")
        wk_sb = const.tile([128, KD, 128], FP8, tag="wk")
        wv_sb = const.tile([128, KD, 128], FP8, tag="wv")
        nc.sync.dma_start(wq_sb[:], wq_d.ap())
        nc.sync.dma_start(wk_sb[:], wk_d.ap())
        bq_sb = const.tile([128, 1], F32, tag="bq")
        bk_sb = const.tile([128, 1], F32, tag="bk")
        xt_sb = persist.tile([128, KD, T], FP8, tag="xt")
        nc.sync.dma_start(xt_sb[:, :, 0:512], xt_d.ap()[:, :, 0:512])
        nc.sync.dma_start(wv_sb[:], wv_d.ap())
        nc.sync.dma_start(bq_sb[:], bq_d.ap())
        nc.sync.dma_start(bk_sb[:], bk_d.ap())
        trimask = const.tile([128, 128], FP8)
        nc.sync.dma_start(trimask[:], tm_d.ap())
        for vc in range(1, NQ):
            v_sl = slice(vc * 512, (vc + 1) * 512)
            nc.sync.dma_start(xt_sb[:, :, v_sl], xt_d.ap()[:, :, v_sl])
        # Warm the ACT exp table during the initial DMAs (first real EXP
        # otherwise pays a ~1.3us ACT_TABLE_LOAD on the critical path).
        # Reads ident (small, first DMA in queue) so it runs immediately.
        warm = const.tile([128, 1], F32, tag="warm")
        nc.scalar.activation(out=warm[:], in_=ident[:, 0:1], func=EXP)
        # Ones row for the PE-based denominator broadcast (gpsimd
        # partition_broadcast stalls ~6-8us on library reload / sem polling).
        ones_r = const.tile([1, 128], BF16, tag="ones_r")
        nc.vector.memset(ones_r[:], 1.0)
        # Warm-up matmuls on ident while the xt/weight DMAs stream: the PE
        # is idle here anyway, and ramping HAM early makes the first proj
        # matmuls run at full clock instead of the cold ~0.65GHz.
        for w in range(24):
            wps = s_ps.tile([128, 128], F32, tag="s", name=f"warm_mm_{w}")
            nc.tensor.matmul(wps[:], ident[:], ident[:], start=True, stop=True)

        # V in fp8 [k-token part, kt, 144(pad for DoubleRow step%16)]: per
        # head 64 V cols + a ones col (65th) that makes the PV matmul emit
        # the softmax denominator. (V bias is folded into kernel B's xb on
        # the host: softmax weights sum to one, so a constant v-offset
        # shifts the output by a constant.)
        v_sb = persist.tile([128, NT, 144], FP8, tag="v")
        nc.gpsimd.memset(v_sb[:, :, 64:65], 1.0)
        nc.gpsimd.memset(v_sb[:, :, 129:130], 1.0)
        qt_sb = persist.tile([128, T], BF16, tag="qt")
        kt_sb = persist.tile([128, T], BF16, tag="kt")
        at_sb = persist.tile([128, T], FP8, tag="at")

        def proj_chains(vc):
            """QKV projection (fp8 DoubleRow over kd-pairs; weights are x64
            host-scaled against fp8 subnormals, un-done by the ACT scale) +
            V-transpose for chunk vc, as 4 separately-emittable chains that
            get spread through the previous chunk's attention steps: the PE
            queue is in-order, so interleaved proj work fills exp-wait slots
            and keeps the HAM p-state high. PSUM tiles borrow score slots."""
            c_sl = slice(vc * 512, (vc + 1) * 512)
            vt_cell = []

            def qk_chain(nm, w_sb, b_sb, o_sb):
                def f():
                    pps = s_ps.tile([128, 512], F32, tag="s",
                                    name=f"pps_{nm}_{vc}")
                    for j in range(KD // 2):
                        nc.tensor.matmul(pps[:], w_sb[:, 2 * j:2 * j + 2, :],
                                         xt_sb[:, 2 * j:2 * j + 2, c_sl],
                                         start=(j == 0),
                                         stop=(j == KD // 2 - 1),
                                         perf_mode=DR)
                    nc.scalar.activation(out=o_sb[:, c_sl], in_=pps[:],
                                         func=IDENT_FN, bias=b_sb[:],
                                         scale=1.0 / 64.0)
                return f

            def v_chain():
                vps = s_ps.tile([128, 512], F32, tag="s", name=f"vps_{vc}")
                for j in range(KD // 2):
                    nc.tensor.matmul(vps[:], wv_sb[:, 2 * j:2 * j + 2, :],
                                     xt_sb[:, 2 * j:2 * j + 2, c_sl],
                                     start=(j == 0), stop=(j == KD // 2 - 1),
                                     perf_mode=DR)
                vt_c = vtp.tile([128, 512], BF16, tag="vt", name=f"vt_{vc}")
                nc.scalar.activation(out=vt_c[:], in_=vps[:], func=IDENT_FN,
                                     scale=1.0 / 64.0)
                vt_cell.append(vt_c)

            def tr_chain():
                vt_c = vt_cell[0]
                tpv = s_ps.tile([128, 4, 128], BF16, tag="s", name=f"tpv_{vc}")
                for q in range(4):
                    nc.tensor.transpose(tpv[:, q, :],
                                        vt_c[:, q * 128:(q + 1) * 128],
                                        ident[:])
                nc.vector.tensor_copy(v_sb[:, vc * 4:(vc + 1) * 4, 0:64],
                                      tpv[:, :, 0:64])
                nc.vector.tensor_copy(v_sb[:, vc * 4:(vc + 1) * 4, 65:129],
                                      tpv[:, :, 64:128])

            return [qk_chain("q", wq_sb, bq_sb, qt_sb),
                    qk_chain("k", wk_sb, bk_sb, kt_sb),
                    v_chain, tr_chain]

        def emit_proj(vc):
            for f in proj_chains(vc):
                f()

        def mask_cols(esb, kt, qc):
            """Causal mask for diagonal k-tile kt on the [128, 1024] fp8 exp
            tile (cols 512h..: head h): zero fully-masked leading columns,
            multiply the 128-wide diagonal block by the triangular mask."""
            o = kt * 128 - qc * 512
            for h in (0, 1):
                if o > 0:
                    nc.gpsimd.memset(esb[:, 512 * h:512 * h + o], 0.0)
                nc.gpsimd.tensor_mul(esb[:, 512 * h + o:512 * h + o + 128],
                                     esb[:, 512 * h + o:512 * h + o + 128],
                                     trimask[:])

        emit_proj(0)
        for qc in range(NQ):
            c_sl = slice(qc * 512, (qc + 1) * 512)
            # ---- attention for chunk qc ----
            nkt = 4 * (qc + 1)
            npair = nkt // 2
            q_sl = c_sl
            pv = [pv_ps.tile([65, 512], F32, tag=f"pv{h}", name=f"pv{h}_{qc}")
                  for h in (0, 1)]

            # Pair-preserving interleave: PV runs as fp8 DoubleRow matmuls
            # over adjacent k-tile pairs (2i, 2i+1), so pairs stay intact.
            # The 2 masked diagonal pairs are spread early among the unmasked
            # ones (u0 d0 u1 d1 u2 u3 ...): their long exp->gpsimd-mask
            # latency hides behind unmasked PV work instead of draining the
            # pipeline at the chunk tail.
            dpairs = [npair - 2, npair - 1]
            upairs = list(range(npair - 2))
            pseq = []
            for i in range(2):
                if i < len(upairs):
                    pseq.append(upairs[i])
                pseq.append(dpairs[i])
            pseq += upairs[2:]
            seq = [2 * p + s for p in pseq for s in (0, 1)]

            n_emitted = 0

            def emit_pv(pair, esb):
                """One DoubleRow matmul per head covers k-tiles 2p and 2p+1:
                lhsT [128, 2, 65] fp8 (two V tiles), rhs [128, 2, 512] fp8
                (two exp tiles interleaved in the free dim)."""
                nonlocal n_emitted
                for h in (0, 1):
                    nc.tensor.matmul(pv[h][:, :],
                                     v_sb[:, 2 * pair:2 * pair + 2,
                                          65 * h:65 * h + 65],
                                     esb[:, :, 512 * h:512 * h + 512],
                                     start=(n_emitted == 0),
                                     stop=(n_emitted == npair - 1),
                                     perf_mode=DR,
                                     skip_group_check=True)
                n_emitted += 1

            esbs = {}
            done_pairs = []
            n_exp = 0
            for j, kt in enumerate(seq):
                if j == 2 and qc + 1 < NQ:
                    emit_proj(qc + 1)
                pair, slot = kt // 2, kt % 2
                is_diag = kt >= nkt - 4
                sp = s_ps.tile([128, 1024], F32, tag="s", name=f"s_{qc}_{kt}")
                for h in (0, 1):
                    h_sl = slice(64 * h, 64 * h + 64)
                    nc.tensor.matmul(sp[:, 512 * h:512 * h + 512],
                                     kt_sb[h_sl, kt * 128:(kt + 1) * 128],
                                     qt_sb[h_sl, q_sl],
                                     start=True, stop=True)
                if slot == 0:
                    esbs[pair] = e_pool.tile([128, 2, 1024], FP8, tag="e",
                                             name=f"e_{qc}_{pair}")
                esb = esbs[pair]
                # Whole-step exp alternating between ACT (exact exp, ~1.03us
                # for 1024 cols) and DVE (Schraudolph fp8 bit-trick, ~1.2us):
                # one instruction per step amortizes the fixed access
                # overhead, ~25% more exp throughput than per-half splitting.
                # (GpSimd cannot read PSUM, so it can't take exp steps.)
                n_exp += 1
                if n_exp % 2 == 1:
                    nc.scalar.activation(out=esb[:, slot, :], in_=sp[:],
                                         func=EXP)
                else:
                    nc.vector.tensor_scalar(out=esb[:, slot, :].bitcast(I8),
                                            in0=sp[:],
                                            scalar1=SCH_A8, scalar2=SCH_B8,
                                            op0=mybir.AluOpType.mult,
                                            op1=mybir.AluOpType.add)
                if is_diag:
                    mask_cols(esb[:, slot, :], kt, qc)
                # after finishing pair m (odd positions), emit PV for the
                # previous pair (one-pair lag to keep the pipeline deep)
                if slot == 1:
                    done_pairs.append(pair)
                    if len(done_pairs) > 2:
                        p = done_pairs.pop(0)
                        emit_pv(p, esbs.pop(p))
            for p in done_pairs:
                emit_pv(p, esbs.pop(p))

            # Evacuate pv immediately (r1 on ACT, dims on DVE) so the
            # single-buffered pv banks free up for the next chunk's PVs.
            for h in (0, 1):
                r1 = rb_pool.tile([1, 512], BF16, tag="r1", name=f"r1{h}_{qc}")
                nc.scalar.copy(r1[:], pv[h][64:65, :])
                pvc = rb_pool.tile([64, 512], F32, tag="pvc", name=f"pvc{h}_{qc}")
                nc.vector.tensor_copy(pvc[:], pv[h][0:64, :])
                rb = s_ps.tile([128, 512], F32, tag="s", name=f"rb{h}_{qc}")
                nc.tensor.matmul(rb[:], ones_r[:], r1[:], start=True, stop=True)
                rbs = rb_pool.tile([128, 512], F32, tag="rbs", name=f"rbs{h}_{qc}")
                nc.vector.reciprocal_approx_fast(out=rbs[:], in_=rb[:])
                nc.vector.tensor_mul(at_sb[64 * h:64 * h + 64, q_sl],
                                     pvc[:], rbs[0:64, :])
            nc.sync.dma_start(at_d.ap()[:, q_sl], at_sb[:, q_sl])

    nc.compile()
    return nc


def build_kernel_b(T=T_FULL):
    """Per core: rows slice of T/8 tokens: out-proj (bf16) + residual (+bout
    folded on host into xb, sent bf16) + LayerNorm*gamma+beta. DMAs are split
    per kd-tile and interleaved so the first matmul starts ~2us in; z is kept
    bf16 (2x DVE rate), LN stats fp32, the (z-mu)*rstd normalize runs on the
    ACT engine (scale/bias APs), gamma/beta loaded [1,D] and broadcast via a
    PE ones-matmul. gpsimd untouched (its teardown drain costs ~50us)."""
    nc = bacc.Bacc("TRN2", target_bir_lowering=False, debug=False)
    Tc = T // NCORES
    KD = D // 128

    at_d = nc.dram_tensor("at", [128, KD, Tc], FP8, kind="ExternalInput")
    wo_d = nc.dram_tensor("wout_t", [128, KD, D], FP8, kind="ExternalInput")
    xb_d = nc.dram_tensor("xb", [Tc, D], BF16, kind="ExternalInput")
    g_d = nc.dram_tensor("gamma", [1, D], BF16, kind="ExternalInput")
    be_d = nc.dram_tensor("beta", [1, D], BF16, kind="ExternalInput")
    y_d = nc.dram_tensor("y", [Tc, D], F32, kind="ExternalOutput")

    with tile.TileContext(nc) as tc, ExitStack() as ctx:
        const = ctx.enter_context(tc.tile_pool(name="const", bufs=1))
        work = ctx.enter_context(tc.tile_pool(name="work", bufs=2))
        stats = ctx.enter_context(tc.tile_pool(name="stats", bufs=4))
        ps = ctx.enter_context(tc.tile_pool(name="ps", bufs=4, space="PSUM"))
        gb_ps = ctx.enter_context(tc.tile_pool(name="gb_ps", bufs=2, space="PSUM"))

        # Whole-tensor DMAs on separate engine queues: small kd-split DMAs
        # serialized descriptor generation on the sync sequencer and delayed
        # the first matmul by ~6us.
        at_sb = const.tile([128, KD, Tc], FP8, tag="at")
        wo_sb = const.tile([128, KD, D], FP8, tag="wo")
        gb_sb = const.tile([1, 2, D], BF16, tag="gb")
        nc.sync.dma_start(gb_sb[:, 0, :], g_d.ap())
        nc.sync.dma_start(gb_sb[:, 1, :], be_d.ap())
        nc.sync.dma_start(at_sb[:], at_d.ap())
        nc.scalar.dma_start(wo_sb[:], wo_d.ap())
        eps_sb = const.tile([128, 1], F32, tag="eps")
        # z is x64-scaled (fp8 weights); LN is scale-invariant given eps*64^2
        nc.vector.memset(eps_sb[:], LN_EPS * 4096.0)
        ones_r = const.tile([1, 128], BF16, tag="ones_r")
        nc.vector.memset(ones_r[:], 1.0)
        gam_b = const.tile([128, D], BF16, tag="gam")
        bet_b = const.tile([128, D], BF16, tag="bet")

        def emit_gb_broadcast():
            # broadcast gamma/beta [1,D] -> [128,D] via PE ones-matmul;
            # emitted after tile 0's matmuls so it doesn't hog the cold PE
            for src, dst in ((0, gam_b), (1, bet_b)):
                for j in (0, 1):
                    gp = gb_ps.tile([128, 512], F32, tag="gp",
                                    name=f"gp_{src}_{j}")
                    nc.tensor.matmul(gp[:], ones_r[:],
                                     gb_sb[:, src, j * 512:(j + 1) * 512],
                                     start=True, stop=True)
                    nc.vector.tensor_copy(dst[:, j * 512:(j + 1) * 512], gp[:])

        for tt in range(Tc // 128):
            t_sl = slice(tt * 128, (tt + 1) * 128)
            xb_t = work.tile([128, D], BF16, tag="xb")
            nc.scalar.dma_start(xb_t[:], xb_d.ap()[t_sl, :])
            z_t = work.tile([128, D], BF16, tag="z")
            st = stats.tile([128, 2, 6], F32, tag="st")
            for j in (0, 1):
                pp = ps.tile([128, 512], F32, tag="pp")
                for kt in range(KD // 2):
                    nc.tensor.matmul(pp[:], at_sb[:, 2 * kt:2 * kt + 2, t_sl],
                                     wo_sb[:, 2 * kt:2 * kt + 2,
                                           j * 512:(j + 1) * 512],
                                     start=(kt == 0), stop=(kt == KD // 2 - 1),
                                     perf_mode=DR)
                nc.vector.tensor_add(z_t[:, j * 512:(j + 1) * 512], pp[:],
                                     xb_t[:, j * 512:(j + 1) * 512])
                nc.vector.bn_stats(st[:, j, :], z_t[:, j * 512:(j + 1) * 512])
            if tt == 0:
                emit_gb_broadcast()
            mv = stats.tile([128, 2], F32, tag="mv")
            nc.vector.bn_aggr(mv[:], st[:])
            sq = stats.tile([128, 1], F32, tag="sq")
            nc.scalar.activation(out=sq[:], in_=mv[:, 1:2], func=SQRT,
                                 bias=eps_sb[:], scale=1.0)
            rstd = stats.tile([128, 1], F32, tag="rstd")
            nc.vector.reciprocal(rstd[:], sq[:])
            nmu = stats.tile([128, 1], F32, tag="nmu")
            nc.vector.tensor_scalar(out=nmu[:], in0=mv[:, 0:1], scalar1=rstd[:],
                                    scalar2=-1.0, op0=mybir.AluOpType.mult,
                                    op1=mybir.AluOpType.mult)
            # (z - mu) * rstd on ACT: func(in*scale + bias)
            zn_t = work.tile([128, D], BF16, tag="zn")
            nc.scalar.activation(out=zn_t[:], in_=z_t[:], func=IDENT_FN,
                                 bias=nmu[:], scale=rstd[:])
            y_t = work.tile([128, D], F32, tag="y")
            nc.vector.tensor_mul(zn_t[:], zn_t[:], gam_b[:])
            nc.vector.tensor_add(y_t[:], zn_t[:], bet_b[:])
            nc.sync.dma_start(y_d.ap()[t_sl, :], y_t[:])

    nc.compile()
    return nc


def _get_kernels(T=T_FULL):
    if T not in _CACHE:
        _CACHE[T] = (build_kernel_a(T), build_kernel_b(T))
    return _CACHE[T]


def _tile_kd(a):
    """[D, M] -> [128, D//128, M] with row = kt*128 + p."""
    Dd, M = a.shape
    return np.ascontiguousarray(a.reshape(Dd // 128, 128, M).transpose(1, 0, 2))


def kernel(x, Wqkv, bqkv, Wout, bout, gamma, beta):
    x = np.asarray(x, dtype=np.float32)
    Wqkv = np.asarray(Wqkv, dtype=np.float32)
    bqkv = np.asarray(bqkv, dtype=np.float32)
    Wout = np.asarray(Wout, dtype=np.float32)
    bout = np.asarray(bout, dtype=np.float32)
    gamma = np.asarray(gamma, dtype=np.float32)
    beta = np.asarray(beta, dtype=np.float32)

    B, T, D_ = x.shape
    assert B == 1 and D_ == D
    d = D // HEADS
    scale = d ** -0.5
    x2d = np.ascontiguousarray(x[0])
    ident = np.eye(128, dtype=np.float32).astype(BF)
    tri = np.triu(np.ones((128, 128), np.float32)).astype(ml_dtypes.float8_e4m3fn)

    nc_a, nc_b = _get_kernels(T)

    F8 = ml_dtypes.float8_e4m3fn

    def q8(a):
        return np.clip(a, -240, 240).astype(F8)

    xt = _tile_kd(q8(x2d.T))                   # [128, 8, T] fp8
    in_maps_a = []
    for c in range(NCORES):
        r = slice(c * 128, (c + 1) * 128)
        wq = Wqkv[0 * D:1 * D][r]
        wk = Wqkv[1 * D:2 * D][r] * scale
        wv = Wqkv[2 * D:3 * D][r]
        in_maps_a.append({
            "xt": xt,
            "ident": ident,
            "trimask": tri,
            # x64 against fp8 subnormals; undone by the ACT 1/64 scale
            "wq_t": _tile_kd(q8(wq.T * 64.0)),
            "wk_t": _tile_kd(q8(wk.T * 64.0)),
            "wv_t": _tile_kd(q8(wv.T * 64.0)),
            "bq": np.ascontiguousarray(bqkv[0 * D:1 * D][r].reshape(128, 1)),
            "bk": np.ascontiguousarray((bqkv[1 * D:2 * D][r] * scale).reshape(128, 1)),
        })
    res_a = run_bass_kernel_spmd(nc_a, in_maps_a, core_ids=list(range(NCORES)))
    LAST_RESULTS["a"] = res_a
    at_full = np.concatenate([np.asarray(res_a.results[c]["at_out"])
                              for c in range(NCORES)], axis=0)  # [D, T] bf16

    Tc = T // NCORES
    wo_tiled = _tile_kd(q8(Wout.T * 64.0))     # [128, 8, D] fp8, x64-scaled
    gam_r = np.ascontiguousarray(gamma.reshape(1, D).astype(BF))
    bet_r = np.ascontiguousarray(beta.reshape(1, D).astype(BF))
    # residual + out-proj bias + folded V bias (constant shift of attention
    # out); x64 to match the fp8 weight scaling (LN un-scales exactly)
    xb_add = bout + Wout @ bqkv[2 * D:3 * D]
    in_maps_b = []
    for c in range(NCORES):
        t_sl = slice(c * Tc, (c + 1) * Tc)
        at_c = at_full[:, t_sl]                # [D, Tc] fp8
        in_maps_b.append({
            "at": _tile_kd(at_c),
            "wout_t": wo_tiled,
            "xb": np.ascontiguousarray(
                ((x2d[t_sl] + xb_add[None, :]) * 64.0).astype(BF)),
            "gamma": gam_r,
            "beta": bet_r,
        })
    res_b = run_bass_kernel_spmd(nc_b, in_maps_b, core_ids=list(range(NCORES)))
    LAST_RESULTS["b"] = res_b
    y = np.concatenate([res_b.results[c]["y"] for c in range(NCORES)], axis=0)
    return y.reshape(1, T, D).astype(np.float32)

